# revision 16
# baseline (speedup 1.0000x reference)
"""NCN link predictor (nn_NCNPredictor_77292231459355) on 8 Trainium2 cores.

Strategy (B-sharded per the sharding hint): the 1024 target pairs are split
128 per core. The host symmetrizes edge_index and re-shards it by target row
(the natural CSR shard for a B-partition): each core receives the padded
adjacency rows of ITS 128 (i, j) target pairs. On device, each core:
  1. computes c[b,q] = multiplicity of j-neighbor q in i's row via a
     broadcast equality + grouped reduce (the A_i*A_j intersection),
  2. compacts the (extremely sparse) nonzero weights with a top-8 pass on
     the packed key c*2^17 + neighbor_id,
  3. gathers only the needed rows of x from HBM via indirect DMA,
  4. computes xcn = sum_k w_k * x[n_k], xij = x[i]*x[j], and the MLP head.
Host concatenates the 8 per-core [128] score slices into the final [1024].
"""

import numpy as np

N_NODES = 100000
B = 1024
D = 128
DH = 512
N_CORES = 8
BL = B // N_CORES  # 128 pairs per core = SBUF partition dim
TOPK = 8

# tuning flags
BF16_EQ = True     # eq matrix in bf16 (exact: values 0/1, sums <= si < 256)
GP_FRAC = 0.0      # Pool engine rejects TensorTensor in codegen; keep 0

_compiled_cache: dict = {}


def _padded_rows(src, dst, targets, sentinel):
    """Padded adjacency rows (with multiplicity as repeated entries) of the
    symmetric edge list at `targets` -> float32 [B, S] (S = max degree,
    padded to a multiple of 8, >= 8). Pad slots hold `sentinel`."""
    b = targets.shape[0]
    pos = np.full(N_NODES, -1, np.int32)
    pos[targets] = np.arange(b, dtype=np.int32)
    r = pos[src]
    m = r >= 0
    rows = r[m].astype(np.int64)
    cols = dst[m].astype(np.int64)
    order = np.argsort(rows, kind="stable")
    rows = rows[order]
    cols = cols[order]
    cnt = np.bincount(rows, minlength=b)
    s = max(8, (int(cnt.max()) + 7) // 8 * 8)
    starts = np.zeros(b + 1, np.int64)
    np.cumsum(cnt, out=starts[1:])
    within = np.arange(rows.size, dtype=np.int64) - starts[rows]
    out = np.full((b, s), sentinel, np.float32)
    out[rows, within] = cols.astype(np.float32)
    return out


def _big_layout(si, sj):
    """Column offsets for the two merged [128, W] per-core input blocks:
    `nin` (small, gates the equality pass) and `wts` (weights, needed late)."""
    lay = {}
    off = 0
    for name, w in [("ni", si), ("nj", sj), ("tij", 2), ("b2b", 1)]:
        lay[name] = ("nin", off, w)
        off += w
    nin_w = off
    off = 0
    for name, w in [("ident", BL), ("b1b", DH), ("w2b", DH),
                    ("w1a", DH), ("w1b", DH)]:
        lay[name] = ("wts", off, w)
        off += w
    return lay, nin_w, off


def _build_bass(si, sj, total_slots, repeat=1):
    """repeat>1 unrolls the whole body N times over the same tiles (serial
    via WAW deps) — used only for amplified wall-clock timing."""
    import concourse.bass as bass
    import concourse.tile as tile
    from concourse import bacc, mybir

    f32 = mybir.dt.float32
    bf16 = mybir.dt.bfloat16
    i32 = mybir.dt.int32
    eq_dt = bf16 if BF16_EQ else f32

    lay, ninw, wtsw = _big_layout(si, sj)
    # q-range split between GPSIMD and DVE for the equality pass
    qg = min(sj - 8, max(0, int(round(sj * GP_FRAC / 8.0)) * 8))

    nc = bacc.Bacc(
        "TRN2", target_bir_lowering=False, debug=False, num_devices=N_CORES
    )

    nin_d = nc.dram_tensor("nin", [BL, ninw], f32, kind="ExternalInput").ap()
    wts_d = nc.dram_tensor("wts", [BL, wtsw], f32, kind="ExternalInput").ap()
    x_d = nc.dram_tensor("x", [N_NODES, D], f32, kind="ExternalInput").ap()
    out_d = nc.dram_tensor("out", [BL, 1], f32, kind="ExternalOutput").ap()

    with tile.TileContext(nc) as tc:
        with (
            tc.tile_pool(name="sb", bufs=2) as sb,
            tc.tile_pool(name="ps", bufs=2, space="PSUM") as ps,
        ):
          for _rep in range(repeat):
            nin = sb.tile([BL, ninw], f32, tag="nin")
            nc.sync.dma_start(nin[:], nin_d[:])
            wts = sb.tile([BL, wtsw], f32, tag="wts")
            nc.sync.dma_start(wts[:], wts_d[:])

            def bslice(name):
                blk, off, w = lay[name]
                t = nin if blk == "nin" else wts
                return t[:, off : off + w]

            nif = bslice("ni")
            njf = bslice("nj")
            tij = bslice("tij").bitcast(i32)
            b2b = bslice("b2b")
            ident = bslice("ident")
            b1b = bslice("b1b")
            w2b = bslice("w2b")
            w1a = bslice("w1a")
            w1b = bslice("w1b")

            # --- xij = x[tar_i] * x[tar_j] (independent; overlaps eq pass) ---
            # NB: HW indirect DMA supports ONE index per partition; a [P,K]
            # index tile gathers K *consecutive* rows from the first index
            # (sim diverges!), so every gather below uses a [P,1] index.
            xs = sb.tile([BL, 2 * D], f32, tag="xs")
            xcn = xs[:, D : 2 * D]
            xi = sb.tile([BL, D], f32, tag="xi")
            xj = sb.tile([BL, D], f32, tag="xj")
            nc.gpsimd.indirect_dma_start(
                out=xi[:], out_offset=None, in_=x_d[:],
                in_offset=bass.IndirectOffsetOnAxis(ap=tij[:, 0:1], axis=0),
            )
            nc.gpsimd.indirect_dma_start(
                out=xj[:], out_offset=None, in_=x_d[:],
                in_offset=bass.IndirectOffsetOnAxis(ap=tij[:, 1:2], axis=0),
            )
            nc.vector.tensor_mul(out=xs[:, 0:D], in0=xi[:], in1=xj[:])

            # --- intersection counts: c[b,q] = sum_p (NJ[b,q] == NI[b,p]).
            # q in [0, qg) on GPSIMD concurrently with q in [qg, sj) on DVE.
            cmat = sb.tile([BL, sj], f32, tag="cmat")
            if qg > 0:
                eq3a = sb.tile([BL, qg * si], eq_dt, tag="eq3a")
                nc.gpsimd.tensor_tensor(
                    out=eq3a[:].rearrange("p (q i) -> p q i", i=si),
                    in0=njf[:, 0:qg].unsqueeze(2).broadcast_to([BL, qg, si]),
                    in1=nif[:].unsqueeze(1).broadcast_to([BL, qg, si]),
                    op=mybir.AluOpType.is_equal,
                )
            qd = sj - qg
            eq3b = sb.tile([BL, qd * si], eq_dt, tag="eq3b")
            nc.vector.tensor_tensor(
                out=eq3b[:].rearrange("p (q i) -> p q i", i=si),
                in0=njf[:, qg:sj].unsqueeze(2).broadcast_to([BL, qd, si]),
                in1=nif[:].unsqueeze(1).broadcast_to([BL, qd, si]),
                op=mybir.AluOpType.is_equal,
            )
            nc.vector.tensor_reduce(
                out=cmat[:, qg:sj],
                in_=eq3b[:].rearrange("p (q i) -> p q i", i=si),
                axis=mybir.AxisListType.X,
                op=mybir.AluOpType.add,
            )
            if qg > 0:
                nc.vector.tensor_reduce(
                    out=cmat[:, 0:qg],
                    in_=eq3a[:].rearrange("p (q i) -> p q i", i=si),
                    axis=mybir.AxisListType.X,
                    op=mybir.AluOpType.add,
                )

            # --- pack keys t = c*2^17 + nj, clamp pads to 0 ---
            tkey = sb.tile([BL, sj], f32, tag="tkey")
            nc.vector.scalar_tensor_tensor(
                out=tkey[:],
                in0=cmat[:],
                scalar=131072.0,
                in1=njf[:],
                op0=mybir.AluOpType.mult,
                op1=mybir.AluOpType.add,
            )
            nc.vector.tensor_scalar_max(out=tkey[:], in0=tkey[:], scalar1=0.0)

            # --- top-8 rounds: decode (w, n), gather x rows, accumulate.
            # Keys sort descending, so positive-weight slots occupy the first
            # `total_slots` columns globally; gather only those. ---
            n_rounds = max(1, -(-total_slots // TOPK))
            first = True
            tk = tkey
            for r in range(n_rounds):
                g = min(TOPK, max(1, total_slots) - r * TOPK)
                t8 = sb.tile([BL, 8], f32, tag=f"t8_{r}")
                nc.vector.max(out=t8[:], in_=tk[:])
                t8i = sb.tile([BL, 8], i32, tag=f"t8i_{r}")
                nc.vector.tensor_copy(out=t8i[:], in_=t8[:])
                n8i = sb.tile([BL, 8], i32, tag=f"n8i_{r}")
                nc.vector.tensor_single_scalar(
                    out=n8i[:], in_=t8i[:], scalar=131071,
                    op=mybir.AluOpType.bitwise_and,
                )
                nc.vector.tensor_single_scalar(
                    out=n8i[:], in_=n8i[:], scalar=N_NODES - 1,
                    op=mybir.AluOpType.min,
                )
                w8i = sb.tile([BL, 8], i32, tag=f"w8i_{r}")
                nc.vector.tensor_single_scalar(
                    out=w8i[:], in_=t8i[:], scalar=17,
                    op=mybir.AluOpType.arith_shift_right,
                )
                w8f = sb.tile([BL, 8], f32, tag=f"w8f_{r}")
                nc.vector.tensor_copy(out=w8f[:], in_=w8i[:])

                for k in range(g):
                    xsel = sb.tile([BL, D], f32, tag=f"xsel_{r}_{k}")
                    nc.gpsimd.indirect_dma_start(
                        out=xsel[:], out_offset=None, in_=x_d[:],
                        in_offset=bass.IndirectOffsetOnAxis(
                            ap=n8i[:, k : k + 1], axis=0
                        ),
                    )
                    if first:
                        nc.vector.tensor_scalar_mul(
                            out=xcn, in0=xsel[:], scalar1=w8f[:, k : k + 1]
                        )
                        first = False
                    else:
                        nc.vector.scalar_tensor_tensor(
                            out=xcn,
                            in0=xsel[:],
                            scalar=w8f[:, k : k + 1],
                            in1=xcn,
                            op0=mybir.AluOpType.mult,
                            op1=mybir.AluOpType.add,
                        )
                if r + 1 < n_rounds:
                    tk2 = sb.tile([BL, sj], f32, tag=f"tkey_{r + 1}")
                    nc.vector.match_replace(
                        out=tk2[:], in_to_replace=t8[:], in_values=tk[:],
                        imm_value=0.0,
                    )
                    tk = tk2

            # --- MLP head: out = relu(xs @ W1 + b1) @ W2 + b2 ---
            pst0 = ps.tile([BL, BL], f32, tag="pst0")
            pst1 = ps.tile([BL, BL], f32, tag="pst1")
            nc.tensor.transpose(out=pst0[:], in_=xs[:, 0:D], identity=ident)
            nc.tensor.transpose(out=pst1[:], in_=xs[:, D : 2 * D], identity=ident)
            xst0 = sb.tile([BL, BL], f32, tag="xst0")
            xst1 = sb.tile([BL, BL], f32, tag="xst1")
            nc.scalar.copy(out=xst0[:], in_=pst0[:])
            nc.scalar.copy(out=xst1[:], in_=pst1[:])

            psh = ps.tile([BL, DH], f32, tag="psh")
            nc.scalar.copy(out=psh[:], in_=b1b)
            nc.tensor.matmul(
                psh[:], lhsT=xst0[:], rhs=w1a,
                start=False, stop=False, skip_group_check=True,
            )
            nc.tensor.matmul(
                psh[:], lhsT=xst1[:], rhs=w1b,
                start=False, stop=True, skip_group_check=True,
            )
            h = sb.tile([BL, DH], f32, tag="h")
            nc.scalar.activation(
                out=h[:], in_=psh[:], func=mybir.ActivationFunctionType.Relu
            )

            # (tensor_tensor_reduce crashes the device on this HW; use a DVE
            # mul then an ACT pass whose accum_out sums the free dim)
            scratch = sb.tile([BL, DH], f32, tag="scratch")
            nc.vector.tensor_mul(out=scratch[:], in0=h[:], in1=w2b)
            dump = sb.tile([BL, DH], f32, tag="dump")
            acc = sb.tile([BL, 1], f32, tag="acc")
            nc.scalar.activation(
                out=dump[:], in_=scratch[:],
                func=mybir.ActivationFunctionType.Copy, accum_out=acc[:],
            )
            res = sb.tile([BL, 1], f32, tag="res")
            nc.scalar.activation(
                out=res[:], in_=acc[:],
                func=mybir.ActivationFunctionType.Identity, bias=b2b,
            )
            nc.sync.dma_start(out_d[:], res[:])

    nc.compile()
    return nc


def _prepare(x, edge_index, tar_ei, W1, b1, W2, b2):
    e0 = np.asarray(edge_index[0]).astype(np.int64)
    e1 = np.asarray(edge_index[1]).astype(np.int64)
    src = np.concatenate([e0, e1])
    dst = np.concatenate([e1, e0])
    tar_i = np.asarray(tar_ei[0]).astype(np.int64)
    tar_j = np.asarray(tar_ei[1]).astype(np.int64)

    ni = _padded_rows(src, dst, tar_i, sentinel=-1.0)
    nj = _padded_rows(src, dst, tar_j, sentinel=-2.0)
    si, sj = ni.shape[1], nj.shape[1]
    assert si <= 127 and sj <= 16384, (si, sj)

    # Safety sizing: rounds of top-8 needed to cover every pair's count of
    # nonzero-weight j-slots (pure planning; the device recomputes all of it).
    eq = nj[:, :, None] == ni[:, None, :]
    total_slots = max(1, int(eq.any(-1).sum(-1).max()))

    x = np.ascontiguousarray(np.asarray(x, dtype=np.float32))
    w1 = np.asarray(W1, dtype=np.float32)
    tij = np.stack([tar_i, tar_j], axis=1).astype(np.int32)

    lay, ninw, wtsw = _big_layout(si, sj)
    blocks = {"nin": np.zeros((B, ninw), np.float32),
              "wts": np.zeros((B, wtsw), np.float32)}

    def put(name, val):
        blk, off, w = lay[name]
        blocks[blk][:, off : off + w] = val

    put("ni", ni)
    put("nj", nj)
    put("tij", tij.view(np.float32))
    put("b2b", np.float32(np.asarray(b2).reshape(-1)[0]))
    put("ident", np.tile(np.eye(BL, dtype=np.float32), (N_CORES, 1)))
    put("b1b", np.asarray(b1, np.float32)[None, :])
    put("w2b", np.asarray(W2, np.float32).reshape(1, DH))
    put("w1a", np.tile(w1[0:D], (N_CORES, 1)))
    put("w1b", np.tile(w1[D : 2 * D], (N_CORES, 1)))

    in_maps = []
    for ci in range(N_CORES):
        sl = slice(ci * BL, (ci + 1) * BL)
        in_maps.append({
            "nin": np.ascontiguousarray(blocks["nin"][sl]),
            "wts": np.ascontiguousarray(blocks["wts"][sl]),
            "x": x,
        })
    return in_maps, si, sj, total_slots


def kernel(x, edge_index, tar_ei, W1, b1, W2, b2):
    from concourse.bass_utils import run_bass_kernel_spmd

    in_maps, si, sj, total_slots = _prepare(x, edge_index, tar_ei, W1, b1, W2, b2)

    key = (si, sj, total_slots)
    if key not in _compiled_cache:
        _compiled_cache[key] = _build_bass(si, sj, total_slots)
    nc = _compiled_cache[key]

    res = run_bass_kernel_spmd(nc, in_maps, list(range(N_CORES)))
    return np.concatenate(
        [res.results[ci]["out"].reshape(BL) for ci in range(N_CORES)]
    ).astype(np.float32)
